# revision 1
# baseline (speedup 1.0000x reference)
"""AttnDecoderRNN teacher-forced decode on 8 TRN2 NeuronCores.

Strategy: the GRU/attention recurrence (small, sequential) is replicated on
every core in a transposed ("T-major": feature-on-partition, batch-on-free)
layout; the dominant output projection h @ out_W.T is vocab-sharded 8 ways
(out_W rows split), so there is no cross-core communication at all.
Per step everything is computed with TensorE matmuls in bf16 (fp32 state,
fp32 PSUM accumulation); the [T*B, V/8] output projection runs batched over
all 48 steps in float32r at full PE rate.
"""

from contextlib import nullcontext
import numpy as np
import ml_dtypes

import concourse.bacc as bacc
import concourse.tile as tile
from concourse.masks import make_identity
import concourse.mybir as mybir

H = 512
L = 64
V = 32000
B = 32
T = 48
NCORES = 8
VS = V // NCORES          # 4000 vocab rows per core
SOS = 1
KH = H // 128             # 4 K-chunks over H
TB = T * B                # 1536
NMT = TB // 128           # 12 output-projection M-tiles
NCH = 8                   # N-chunks of 500 for the projection
NCK = VS // NCH           # 500

f32 = mybir.dt.float32
f32r = mybir.dt.float32r
bf16 = mybir.dt.bfloat16
AF = mybir.ActivationFunctionType

_CACHE: dict = {}


def _pack_kM(wT: np.ndarray, nk: int, nm: int) -> np.ndarray:
    """[nk*128, nm*128] -> [128, nk, nm, 128] stationary-tile layout."""
    return np.ascontiguousarray(
        wT.reshape(nk, 128, nm, 128).transpose(1, 0, 2, 3))


def _pack_k(wT: np.ndarray, nk: int) -> np.ndarray:
    """[nk*128, N] -> [128, nk, N]."""
    n = wT.shape[1]
    return np.ascontiguousarray(wT.reshape(nk, 128, n).transpose(1, 0, 2))


def _build():
    nc = bacc.Bacc("TRN2", target_bir_lowering=False, debug=False)

    def din(name, shape, dt):
        return nc.dram_tensor(name, shape, dt, kind="ExternalInput").ap()

    d_embT = din("embT", [128, KH, TB], bf16)
    d_WeT = din("WeT", [128, KH, L], bf16)
    d_WhT = din("WhT", [128, KH, L], bf16)
    d_combT = din("combT", [128, 2 * KH, KH, 128], bf16)
    d_WihT = din("WihT", [128, KH, 3 * KH, 128], bf16)
    d_WhhT = din("WhhT", [128, KH, 3 * KH, 128], bf16)
    d_encp = din("encp", [128, B // 2, KH, 128], bf16)
    d_outWT = din("outWT", [128, KH, VS], f32r)
    d_h0T32 = din("h0T32", [128, KH, B], f32r)
    d_h0Tbf = din("h0Tbf", [128, KH, B], bf16)
    d_out = nc.dram_tensor("o", [TB, VS], f32, kind="ExternalOutput").ap()
    import os
    _reps = int(os.environ.get("KREPS", "1"))
    _abl = os.environ.get("KABL", "")
    _dbg = bool(int(os.environ.get("KDBG", "0")))
    d_hdbg = (nc.dram_tensor("hdbg", [128, KH, T, B], f32r,
                             kind="ExternalOutput").ap() if _dbg else None)

    with tile.TileContext(nc) as tc:
        with tc.tile_pool(name="con", bufs=1) as con, \
             tc.tile_pool(name="hbfp", bufs=2) as hbfp, \
             tc.tile_pool(name="gw", bufs=2) as gw, \
             tc.tile_pool(name="olog", bufs=2) as ologp, \
             tc.tile_pool(name="psc", bufs=2, space="PSUM") as psc, \
             tc.tile_pool(name="pzz", bufs=1, space="PSUM") as pzz, \
             tc.tile_pool(name="pap", bufs=1, space="PSUM") as pap, \
             tc.tile_pool(name="pcb", bufs=1, space="PSUM") as pcb, \
             tc.tile_pool(name="pg", bufs=1, space="PSUM") as pg, \
             tc.tile_pool(name="plog", bufs=2, space="PSUM") as plog:

            # ---- resident constants ----
            s_embT = con.tile([128, KH, TB], bf16, tag="embT")
            s_WeT = con.tile([128, KH, L], bf16, tag="WeT")
            s_WhT = con.tile([128, KH, L], bf16, tag="WhT")
            s_combT = con.tile([128, 2 * KH, KH, 128], bf16, tag="combT")
            s_WihT = con.tile([128, KH, 3 * KH, 128], bf16, tag="WihT")
            s_WhhT = con.tile([128, KH, 3 * KH, 128], bf16, tag="WhhT")
            s_encp = con.tile([128, B // 2, KH, 128], bf16, tag="encp")
            s_outWT = con.tile([128, KH, VS], f32r, tag="outWT")
            s_h0T32 = con.tile([128, KH, B], f32r, tag="h0T32")
            s_h0Tbf = con.tile([128, KH, B], bf16, tag="h0Tbf")
            for dst, src in [(s_embT, d_embT), (s_WeT, d_WeT), (s_WhT, d_WhT),
                             (s_combT, d_combT), (s_WihT, d_WihT),
                             (s_WhhT, d_WhhT), (s_encp, d_encp),
                             (s_outWT, d_outWT), (s_h0T32, d_h0T32),
                             (s_h0Tbf, d_h0Tbf)]:
                nc.sync.dma_start(out=dst, in_=src)

            s_HT32 = con.tile([128, KH, T, B], f32r, tag="HT32")
            ones128 = con.tile([128, 1], bf16, tag="ones128")
            onesK1 = con.tile([1, 128], f32, tag="onesK1")
            nc.vector.memset(ones128, 1.0)
            nc.vector.memset(onesK1, 1.0)
            masters = [con.tile([128, B // 2, 2], bf16, tag=f"master{i}",
                                name=f"master{i}") for i in range(2)]
            for m in masters:
                nc.vector.memset(m, 0.0)

            with (tc.For_i(0, _reps, 1) if _reps > 1 else nullcontext()):
                prev32 = s_h0T32
                prevbf = s_h0Tbf

                for t in range(T):
                    # ---- attention scores: scT [L, B] (emb part first: it has
                    # no dependence on h, so it can run during the previous
                    # step's tail) ----
                    p_sc = psc.tile([L, B // 2, 2], f32, tag="psc")
                    p_sc_f = p_sc.rearrange("l a b -> l (a b)")
                    for k in range(KH):
                        nc.tensor.matmul(p_sc_f, s_WeT[:, k, :],
                                         s_embT[:, k, B * t:B * (t + 1)],
                                         start=(k == 0), stop=False)
                    for k in range(KH):
                        nc.tensor.matmul(p_sc_f, s_WhT[:, k, :], prevbf[:, k, :],
                                         start=False, stop=(k == KH - 1))

                    # ---- E = exp(scores), written masked into the einsum master ----
                    master = masters[t % 2]
                    nc.scalar.activation(master[0:L, :, 0], p_sc[:, :, 0], AF.Exp)
                    nc.scalar.activation(master[L:128, :, 1], p_sc[:, :, 1], AF.Exp)

                    # ---- unnormalised einsum: appliedT [128, KH, B] ----
                    p_ap = pap.tile([128, KH, B], f32, tag="pap")
                    if _abl == "einsum":
                        nc.vector.memset(p_ap, 0.0)
                    for p in range(0 if _abl != "einsum" else B // 2, B // 2):
                        for c in range(KH):
                            nc.tensor.matmul(p_ap[:, c, 2 * p:2 * p + 2],
                                             s_encp[:, p, c, :], master[:, p, :],
                                             start=True, stop=True)
                    # softmax denominator (from the same bf16 E the einsum uses)
                    p_z = pzz.tile([1, B], f32, tag="pzz")
                    nc.tensor.matmul(p_z, ones128,
                                     master.rearrange("q a b -> q (a b)"),
                                     start=True, stop=True)
                    z_s = gw.tile([1, B], f32, tag="z_s")
                    nc.vector.tensor_copy(z_s, p_z)
                    p_zb = pzz.tile([128, B], f32, tag="pzz")
                    nc.tensor.matmul(p_zb, onesK1, z_s, start=True, stop=True)
                    zb = gw.tile([128, B], f32, tag="zb")
                    nc.vector.reciprocal(zb, p_zb)
                    apbf = gw.tile([128, KH, B], bf16, tag="apbf")
                    nc.vector.tensor_mul(apbf, p_ap,
                                         zb[:, None, :].broadcast_to([128, KH, B]))

                    # ---- comb + relu: xT [128, KH, B] ----
                    p_cb = pcb.tile([128, KH, B], f32, tag="pcb")
                    if _abl == "comb":
                        nc.vector.memset(p_cb, 0.0)
                    for m in range(0 if _abl != "comb" else KH, KH):
                        for k in range(2 * KH):
                            rhs = (s_embT[:, k, B * t:B * (t + 1)] if k < KH
                                   else apbf[:, k - KH, :])
                            nc.tensor.matmul(p_cb[:, m, :], s_combT[:, k, m, :], rhs,
                                             start=(k == 0), stop=(k == 2 * KH - 1))
                    xbf = gw.tile([128, KH, B], bf16, tag="xbf")
                    nc.scalar.activation(xbf, p_cb, AF.Relu)

                    # ---- GRU gate matmuls ----
                    # p_g slots: 0:8 = rz (x- and h- parts accumulated),
                    #            8:12 = xn, 12:16 = hn (h-weights pre-scaled 0.5)
                    p_g = pg.tile([128, 16, B], f32, tag="pg")
                    if _abl == "gru":
                        nc.vector.memset(p_g, 0.0)
                    for m in range(0 if _abl != "gru" else 8, 8):
                        for k in range(KH):
                            nc.tensor.matmul(p_g[:, m, :], s_WihT[:, k, m, :],
                                             xbf[:, k, :], start=(k == 0), stop=False)
                        for k in range(KH):
                            nc.tensor.matmul(p_g[:, m, :], s_WhhT[:, k, m, :],
                                             prevbf[:, k, :], start=False,
                                             stop=(k == KH - 1))
                    for m in range(0 if _abl != "gru" else 4, 4):
                        for k in range(KH):
                            nc.tensor.matmul(p_g[:, 8 + m, :], s_WihT[:, k, 8 + m, :],
                                             xbf[:, k, :], start=(k == 0),
                                             stop=(k == KH - 1))
                    for m in range(0 if _abl != "gru" else 4, 4):
                        for k in range(KH):
                            nc.tensor.matmul(p_g[:, 12 + m, :], s_WhhT[:, k, 8 + m, :],
                                             prevbf[:, k, :], start=(k == 0),
                                             stop=(k == KH - 1))

                    # ---- gate math (fp32) ----
                    # r = sigmoid(s_r) = 0.5 + 0.5*tanh(0.5*s_r)  (tanh shares the
                    # exp table set, avoiding a per-step ACT table swap)
                    t_r = gw.tile([128, KH, B], f32, tag="t_r")
                    nc.scalar.activation(t_r, p_g[:, 0:4, :], AF.Tanh, scale=0.5)
                    t_z = gw.tile([128, KH, B], f32, tag="t_z")
                    nc.scalar.activation(t_z, p_g[:, 4:8, :], AF.Tanh, scale=0.5)
                    # r*hn = hn' + t_r*hn'   with hn' = 0.5*hn
                    u = gw.tile([128, KH, B], f32, tag="u")
                    nc.vector.tensor_mul(u, t_r, p_g[:, 12:16, :])
                    a1 = gw.tile([128, KH, B], f32, tag="a1")
                    nc.vector.tensor_add(a1, u, p_g[:, 8:12, :])
                    narg = gw.tile([128, KH, B], f32, tag="narg")
                    nc.vector.tensor_add(narg, a1, p_g[:, 12:16, :])
                    n_t = gw.tile([128, KH, B], f32, tag="n_t")
                    nc.scalar.activation(n_t, narg, AF.Tanh)
                    # h' = (1-z)n + z h = 0.5*[(h+n) + t_z*(h-n)]
                    d_t = gw.tile([128, KH, B], f32, tag="d_t")
                    nc.vector.tensor_sub(d_t, prev32, n_t)
                    f_t = gw.tile([128, KH, B], f32, tag="f_t")
                    nc.vector.tensor_add(f_t, prev32, n_t)
                    e_t = gw.tile([128, KH, B], f32, tag="e_t")
                    nc.vector.tensor_mul(e_t, t_z, d_t)
                    g2 = gw.tile([128, KH, B], f32, tag="g2")
                    nc.vector.tensor_add(g2, e_t, f_t)
                    nc.vector.tensor_scalar_mul(s_HT32[:, :, t, :], g2, 0.5)
                    hbf = hbfp.tile([128, KH, B], bf16, tag="hbf")
                    nc.scalar.mul(hbf, g2, 0.5)
                    prev32 = s_HT32[:, :, t, :]
                    prevbf = hbf

                    # ---- batched output projection for finished 4-step group ----
                    if t % 4 == 3 and _abl != "phaseB":
                        m = t // 4
                        stg = ologp.tile([128, VS], f32, tag="olog")
                        for j in range(NCH):
                            pt = plog.tile([128, NCK], f32, tag="plog")
                            for k in range(KH):
                                nc.tensor.matmul(
                                    pt,
                                    s_HT32[:, k, 4 * m:4 * (m + 1), :]
                                        .rearrange("q t b -> q (t b)"),
                                    s_outWT[:, k, NCK * j:NCK * (j + 1)],
                                    start=(k == 0), stop=(k == KH - 1))
                                # alternate evacuation engine to spread load
                            if j % 2 == 0:
                                nc.vector.tensor_copy(stg[:, NCK * j:NCK * (j + 1)], pt)
                            else:
                                nc.scalar.copy(stg[:, NCK * j:NCK * (j + 1)], pt)
                        nc.sync.dma_start(out=d_out[128 * m:128 * (m + 1), :], in_=stg)

            if _dbg:
                nc.sync.dma_start(out=d_hdbg, in_=s_HT32)

    nc.compile()
    return nc


def _prep_inputs(inputs):
    enc = np.asarray(inputs["encoded"], np.float32)      # [L, B, H]
    hidden = np.asarray(inputs["hidden"], np.float32)    # [1, B, H]
    target = np.asarray(inputs["target"])                # [T, B] int
    emb = np.asarray(inputs["emb"], np.float32)          # [V, H]
    attn_W = np.asarray(inputs["attn_W"], np.float32)    # [L, 2H]
    comb_W = np.asarray(inputs["comb_W"], np.float32)    # [H, 2H]
    W_ih = np.asarray(inputs["W_ih"], np.float32)        # [3H, H]
    W_hh = np.asarray(inputs["W_hh"], np.float32)        # [3H, H]
    out_W = np.asarray(inputs["out_W"], np.float32)      # [V, H]
    for bname in ("attn_b", "comb_b", "b_ih", "b_hh", "out_b"):
        assert np.abs(np.asarray(inputs[bname])).max() == 0.0, \
            f"nonzero bias {bname} not supported"

    tokens = np.concatenate(
        [np.full((1, B), SOS, target.dtype), target[:-1]], axis=0)  # [T, B]
    emb_seq = emb[tokens.reshape(-1).astype(np.int64)]              # [T*B, H]
    embT = _pack_k(np.ascontiguousarray(emb_seq.T), KH).astype(ml_dtypes.bfloat16)

    WeT = _pack_k(np.ascontiguousarray(attn_W[:, :H].T), KH).astype(ml_dtypes.bfloat16)
    WhT = _pack_k(np.ascontiguousarray(attn_W[:, H:].T), KH).astype(ml_dtypes.bfloat16)
    combT = _pack_kM(np.ascontiguousarray(comb_W.T), 2 * KH, KH).astype(ml_dtypes.bfloat16)
    WihT = _pack_kM(np.ascontiguousarray(W_ih.T), KH, 3 * KH).astype(ml_dtypes.bfloat16)
    W_hh2 = W_hh.copy()
    W_hh2[2 * H:] *= 0.5
    WhhT = _pack_kM(np.ascontiguousarray(W_hh2.T), KH, 3 * KH).astype(ml_dtypes.bfloat16)

    # einsum stationary: encp[(l + 64*half), p, c, m] = enc[l, 2p+half, 128c+m]
    e5 = enc.reshape(L, B // 2, 2, KH, 128)
    encp = np.ascontiguousarray(
        e5.transpose(2, 0, 1, 3, 4).reshape(128, B // 2, KH, 128)
    ).astype(ml_dtypes.bfloat16)

    h0T = np.ascontiguousarray(hidden[0].T)              # [H, B]
    h0T32 = _pack_k(h0T, KH)
    h0Tbf = h0T32.astype(ml_dtypes.bfloat16)

    base = dict(embT=embT, WeT=WeT, WhT=WhT, combT=combT, WihT=WihT,
                WhhT=WhhT, encp=encp, h0T32=h0T32, h0Tbf=h0Tbf)
    in_maps = []
    for c in range(NCORES):
        m = dict(base)
        wc = np.ascontiguousarray(out_W[c * VS:(c + 1) * VS].T)  # [H, VS]
        m["outWT"] = _pack_k(wc, KH)
        in_maps.append(m)
    return in_maps


def _get_runner():
    import os as _os
    _key = ("runner", _os.environ.get("KREPS", "1"), _os.environ.get("KABL", ""),
            _os.environ.get("KDBG", "0"))
    if _key in _CACHE:
        return _CACHE[_key]
    if int(_os.environ.get("KLDW", "0")):
        import concourse.bass_utils as _bu
        if not getattr(_bu, "_ldw_patched", False):
            _orig_rc = _bu.run_command

            def _rc(argv, **kw):
                argv = [a.replace("--enable-ldw-opt=false", "--enable-ldw-opt=true")
                        if isinstance(a, str) else a for a in argv]
                return _orig_rc(argv, **kw)

            _bu.run_command = _rc
            _bu._ldw_patched = True
    import jax
    from jax.sharding import Mesh, PartitionSpec
    try:
        from jax.experimental.shard_map import shard_map
    except ImportError:
        from jax.shard_map import shard_map
    from concourse import bass2jax
    import concourse.mybir as mb

    nc = _build()
    bass2jax.install_neuronx_cc_hook()

    part_name = (nc.partition_id_tensor.name
                 if nc.partition_id_tensor else None)
    in_names, out_names, out_avals, zero_outs = [], [], [], []
    for alloc in nc.m.functions[0].allocations:
        if not isinstance(alloc, mb.MemoryLocationSet):
            continue
        name = alloc.memorylocations[0].name
        if alloc.kind == "ExternalInput":
            if name != part_name:
                in_names.append(name)
        elif alloc.kind == "ExternalOutput":
            out_names.append(name)
            shape = tuple(alloc.tensor_shape)
            dtype = mb.dt.np(alloc.dtype)
            out_avals.append(jax.core.ShapedArray(shape, dtype))
            zero_outs.append(np.zeros(shape, dtype))
    n_params = len(in_names)
    all_names = in_names + out_names
    if part_name is not None:
        all_names = all_names + [part_name]

    def _body(*args):
        operands = list(args)
        if part_name is not None:
            operands.append(bass2jax.partition_id_tensor())
        outs = bass2jax._bass_exec_p.bind(
            *operands,
            out_avals=tuple(out_avals),
            in_names=tuple(all_names),
            out_names=tuple(out_names),
            lowering_input_output_aliases=(),
            sim_require_finite=True,
            sim_require_nnan=True,
            nc=nc,
        )
        return tuple(outs)

    devices = jax.devices()[:NCORES]
    mesh = Mesh(np.asarray(devices), ("core",))
    nin = n_params + len(out_names)
    sharded = jax.jit(
        shard_map(_body, mesh=mesh,
                  in_specs=(PartitionSpec("core"),) * nin,
                  out_specs=(PartitionSpec("core"),) * len(out_names),
                  check_rep=False),
        donate_argnums=tuple(range(n_params, nin)),
        keep_unused=True,
    )

    def runner(in_maps):
        concat_in = [
            np.concatenate([np.asarray(in_maps[c][nm]) for c in range(NCORES)],
                           axis=0)
            for nm in in_names
        ]
        concat_zeros = [
            np.zeros((NCORES * z.shape[0], *z.shape[1:]), z.dtype)
            for z in zero_outs
        ]
        out_arrs = sharded(*concat_in, *concat_zeros)
        o = np.asarray(out_arrs[out_names.index("o")]).reshape(NCORES, TB, VS)
        if "hdbg" in out_names:
            h = np.asarray(out_arrs[out_names.index("hdbg")])
            _CACHE["hdbg"] = h.reshape(NCORES, 128, KH, T, B)[0]
        return o

    _CACHE[_key] = runner
    return runner


def kernel(**inputs) -> np.ndarray:
    in_maps = _prep_inputs(inputs)
    o = _get_runner()(in_maps)               # [NCORES, T*B, VS]
    out = np.concatenate([o[c].reshape(T, B, VS) for c in range(NCORES)],
                         axis=2)             # [T, B, V]
    return np.ascontiguousarray(out, dtype=np.float32)



# revision 5
# speedup vs baseline: 11480.6276x; 11480.6276x over previous
"""AttnDecoderRNN teacher-forced decode on 8 TRN2 NeuronCores.

Strategy: the GRU/attention recurrence (small, sequential) is replicated on
every core in a transposed ("T-major": feature-on-partition, batch-on-free)
layout; the dominant output projection h @ out_W.T is vocab-sharded 8 ways
(out_W rows split), so there is no cross-core communication at all.
Per step everything is computed with TensorE matmuls in bf16 (fp32 state,
fp32 PSUM accumulation); the [T*B, V/8] output projection runs batched over
all 48 steps in float32r at full PE rate.
"""

from contextlib import nullcontext
import numpy as np
import ml_dtypes

import concourse.bacc as bacc
import concourse.tile as tile
from concourse.masks import make_identity
import concourse.mybir as mybir

H = 512
L = 64
V = 32000
B = 32
T = 48
NCORES = 8
VS = V // NCORES          # 4000 vocab rows per core
SOS = 1
KH = H // 128             # 4 K-chunks over H
TB = T * B                # 1536
NMT = TB // 128           # 12 output-projection M-tiles
NCH = 8                   # N-chunks of 500 for the projection
NCK = VS // NCH           # 500

f32 = mybir.dt.float32
f32r = mybir.dt.float32r
bf16 = mybir.dt.bfloat16
fp16 = mybir.dt.float16
AF = mybir.ActivationFunctionType

_CACHE: dict = {}


def _pack_kM(wT: np.ndarray, nk: int, nm: int) -> np.ndarray:
    """[nk*128, nm*128] -> [128, nk, nm, 128] stationary-tile layout."""
    return np.ascontiguousarray(
        wT.reshape(nk, 128, nm, 128).transpose(1, 0, 2, 3))


def _pack_k(wT: np.ndarray, nk: int) -> np.ndarray:
    """[nk*128, N] -> [128, nk, N]."""
    n = wT.shape[1]
    return np.ascontiguousarray(wT.reshape(nk, 128, n).transpose(1, 0, 2))


def _build():
    nc = bacc.Bacc("TRN2", target_bir_lowering=False, debug=False)

    def din(name, shape, dt):
        return nc.dram_tensor(name, shape, dt, kind="ExternalInput").ap()

    d_embT = din("embT", [128, KH, TB], bf16)
    d_WeT = din("WeT", [128, KH, L], bf16)
    d_WhT = din("WhT", [128, KH, L], bf16)
    d_combT = din("combT", [128, 2 * KH, KH, 128], bf16)
    d_WihT = din("WihT", [128, KH, 3 * KH, 128], bf16)
    d_WhhT = din("WhhT", [128, KH, 3 * KH, 128], bf16)
    d_encp = din("encp", [128, B // 2, KH, 128], bf16)
    d_outWT = din("outWT", [128, KH, VS], f32r)
    d_h0T32 = din("h0T32", [128, KH, B], f32r)
    d_h0Tbf = din("h0Tbf", [128, KH, B], bf16)
    d_out = nc.dram_tensor("o", [TB, VS], fp16, kind="ExternalOutput").ap()
    import os
    _reps = int(os.environ.get("KREPS", "1"))
    _abl = os.environ.get("KABL", "")
    _dbg = bool(int(os.environ.get("KDBG", "0")))
    d_hdbg = (nc.dram_tensor("hdbg", [128, KH, T, B], f32r,
                             kind="ExternalOutput").ap() if _dbg else None)

    with tile.TileContext(nc) as tc:
        with tc.tile_pool(name="con", bufs=1) as con, \
             tc.tile_pool(name="hbfp", bufs=2) as hbfp, \
             tc.tile_pool(name="gw", bufs=2) as gw, \
             tc.tile_pool(name="olog", bufs=2) as ologp, \
             tc.tile_pool(name="psc", bufs=2, space="PSUM") as psc, \
             tc.tile_pool(name="pzz", bufs=1, space="PSUM") as pzz, \
             tc.tile_pool(name="pap", bufs=1, space="PSUM") as pap, \
             tc.tile_pool(name="pcb", bufs=1, space="PSUM") as pcb, \
             tc.tile_pool(name="pg", bufs=1, space="PSUM") as pg, \
             tc.tile_pool(name="plog", bufs=2, space="PSUM") as plog:

            # ---- resident constants ----
            s_embT = con.tile([128, KH, TB], bf16, tag="embT")
            s_WeT = con.tile([128, KH, L], bf16, tag="WeT")
            s_WhT = con.tile([128, KH, L], bf16, tag="WhT")
            s_combT = con.tile([128, 2 * KH, KH, 128], bf16, tag="combT")
            s_WihT = con.tile([128, KH, 3 * KH, 128], bf16, tag="WihT")
            s_WhhT = con.tile([128, KH, 3 * KH, 128], bf16, tag="WhhT")
            s_encp = con.tile([128, B // 2, KH, 128], bf16, tag="encp")
            s_outWT = con.tile([128, KH, VS], f32r, tag="outWT")
            s_h0T32 = con.tile([128, KH, B], f32r, tag="h0T32")
            s_h0Tbf = con.tile([128, KH, B], bf16, tag="h0Tbf")
            for dst, src in [(s_embT, d_embT), (s_WeT, d_WeT), (s_WhT, d_WhT),
                             (s_combT, d_combT), (s_WihT, d_WihT),
                             (s_WhhT, d_WhhT), (s_encp, d_encp),
                             (s_outWT, d_outWT), (s_h0T32, d_h0T32),
                             (s_h0Tbf, d_h0Tbf)]:
                nc.sync.dma_start(out=dst, in_=src)

            s_HT32 = con.tile([128, KH, T, B], f32r, tag="HT32")
            ones128 = con.tile([128, 1], bf16, tag="ones128")
            onesK1 = con.tile([1, 128], f32, tag="onesK1")
            nc.vector.memset(ones128, 1.0)
            nc.vector.memset(onesK1, 1.0)
            masters = [con.tile([128, B // 2, 2], bf16, tag=f"master{i}",
                                name=f"master{i}") for i in range(2)]
            for m in masters:
                nc.vector.memset(m, 0.0)

            with (tc.For_i(0, _reps, 1) if _reps > 1 else nullcontext()):
                prev32 = s_h0T32
                prevbf = s_h0Tbf

                for t in range(T):
                    # ---- attention scores: scT [L, B] (emb part first: it has
                    # no dependence on h, so it can run during the previous
                    # step's tail) ----
                    p_sc = psc.tile([L, B // 2, 2], f32, tag="psc")
                    p_sc_f = p_sc.rearrange("l a b -> l (a b)")
                    for k in range(KH):
                        nc.tensor.matmul(p_sc_f, s_WeT[:, k, :],
                                         s_embT[:, k, B * t:B * (t + 1)],
                                         start=(k == 0), stop=False)
                    for k in range(KH):
                        nc.tensor.matmul(p_sc_f, s_WhT[:, k, :], prevbf[:, k, :],
                                         start=False, stop=(k == KH - 1))

                    # ---- E = exp(scores), written masked into the einsum master ----
                    master = masters[t % 2]
                    nc.scalar.activation(master[0:L, :, 0], p_sc[:, :, 0], AF.Exp)
                    nc.scalar.activation(master[L:128, :, 1], p_sc[:, :, 1], AF.Exp)

                    # ---- unnormalised einsum: appliedT [128, KH, B] ----
                    p_ap = pap.tile([128, KH, B], f32, tag="pap")
                    if _abl == "einsum":
                        nc.vector.memset(p_ap, 0.0)
                    for p in range(0 if _abl != "einsum" else B // 2, B // 2):
                        for c in range(KH):
                            nc.tensor.matmul(p_ap[:, c, 2 * p:2 * p + 2],
                                             s_encp[:, p, c, :], master[:, p, :],
                                             start=True, stop=True)
                    # softmax denominator (from the same bf16 E the einsum uses)
                    p_z = pzz.tile([1, B], f32, tag="pzz")
                    nc.tensor.matmul(p_z, ones128,
                                     master.rearrange("q a b -> q (a b)"),
                                     start=True, stop=True)
                    z_s = gw.tile([1, B], f32, tag="z_s")
                    nc.vector.tensor_copy(z_s, p_z)
                    p_zb = pzz.tile([128, B], f32, tag="pzz")
                    nc.tensor.matmul(p_zb, onesK1, z_s, start=True, stop=True)
                    zb = gw.tile([128, B], f32, tag="zb")
                    nc.vector.reciprocal(zb, p_zb)
                    apbf = gw.tile([128, KH, B], bf16, tag="apbf")
                    nc.vector.tensor_mul(apbf, p_ap,
                                         zb[:, None, :].broadcast_to([128, KH, B]))

                    # ---- comb + relu: xT [128, KH, B] ----
                    p_cb = pcb.tile([128, KH, B], f32, tag="pcb")
                    if _abl == "comb":
                        nc.vector.memset(p_cb, 0.0)
                    for m in range(0 if _abl != "comb" else KH, KH):
                        for k in range(2 * KH):
                            rhs = (s_embT[:, k, B * t:B * (t + 1)] if k < KH
                                   else apbf[:, k - KH, :])
                            nc.tensor.matmul(p_cb[:, m, :], s_combT[:, k, m, :], rhs,
                                             start=(k == 0), stop=(k == 2 * KH - 1))
                    xbf = gw.tile([128, KH, B], bf16, tag="xbf")
                    nc.scalar.activation(xbf, p_cb, AF.Relu)

                    # ---- GRU gate matmuls ----
                    # p_g slots: 0:8 = rz (x- and h- parts accumulated),
                    #            8:12 = xn, 12:16 = hn (h-weights pre-scaled 0.5)
                    p_g = pg.tile([128, 16, B], f32, tag="pg")
                    if _abl == "gru":
                        nc.vector.memset(p_g, 0.0)
                    for m in range(0 if _abl != "gru" else 8, 8):
                        for k in range(KH):
                            nc.tensor.matmul(p_g[:, m, :], s_WihT[:, k, m, :],
                                             xbf[:, k, :], start=(k == 0), stop=False)
                        for k in range(KH):
                            nc.tensor.matmul(p_g[:, m, :], s_WhhT[:, k, m, :],
                                             prevbf[:, k, :], start=False,
                                             stop=(k == KH - 1))
                    for m in range(0 if _abl != "gru" else 4, 4):
                        for k in range(KH):
                            nc.tensor.matmul(p_g[:, 8 + m, :], s_WihT[:, k, 8 + m, :],
                                             xbf[:, k, :], start=(k == 0),
                                             stop=(k == KH - 1))
                    for m in range(0 if _abl != "gru" else 4, 4):
                        for k in range(KH):
                            nc.tensor.matmul(p_g[:, 12 + m, :], s_WhhT[:, k, 8 + m, :],
                                             prevbf[:, k, :], start=(k == 0),
                                             stop=(k == KH - 1))

                    # ---- gate math (fp32) ----
                    # r = sigmoid(s_r) = 0.5 + 0.5*tanh(0.5*s_r)  (tanh shares the
                    # exp table set, avoiding a per-step ACT table swap)
                    t_r = gw.tile([128, KH, B], f32, tag="t_r")
                    nc.scalar.activation(t_r, p_g[:, 0:4, :], AF.Tanh, scale=0.5)
                    t_z = gw.tile([128, KH, B], f32, tag="t_z")
                    nc.scalar.activation(t_z, p_g[:, 4:8, :], AF.Tanh, scale=0.5)
                    # r*hn = hn' + t_r*hn'   with hn' = 0.5*hn
                    u = gw.tile([128, KH, B], f32, tag="u")
                    nc.vector.tensor_mul(u, t_r, p_g[:, 12:16, :])
                    a1 = gw.tile([128, KH, B], f32, tag="a1")
                    nc.vector.tensor_add(a1, u, p_g[:, 8:12, :])
                    narg = gw.tile([128, KH, B], f32, tag="narg")
                    nc.vector.tensor_add(narg, a1, p_g[:, 12:16, :])
                    n_t = gw.tile([128, KH, B], f32, tag="n_t")
                    nc.scalar.activation(n_t, narg, AF.Tanh)
                    # h' = (1-z)n + z h = 0.5*[(h+n) + t_z*(h-n)]
                    d_t = gw.tile([128, KH, B], f32, tag="d_t")
                    nc.vector.tensor_sub(d_t, prev32, n_t)
                    f_t = gw.tile([128, KH, B], f32, tag="f_t")
                    nc.vector.tensor_add(f_t, prev32, n_t)
                    e_t = gw.tile([128, KH, B], f32, tag="e_t")
                    nc.vector.tensor_mul(e_t, t_z, d_t)
                    g2 = gw.tile([128, KH, B], f32, tag="g2")
                    nc.vector.tensor_add(g2, e_t, f_t)
                    nc.vector.tensor_scalar_mul(s_HT32[:, :, t, :], g2, 0.5)
                    hbf = hbfp.tile([128, KH, B], bf16, tag="hbf")
                    nc.scalar.mul(hbf, g2, 0.5)
                    prev32 = s_HT32[:, :, t, :]
                    prevbf = hbf

                    # ---- batched output projection for finished 4-step group ----
                    if t % 4 == 3 and _abl != "phaseB":
                        m = t // 4
                        stg = ologp.tile([128, VS], fp16, tag="olog")
                        for j in range(NCH):
                            pt = plog.tile([128, NCK], f32, tag="plog")
                            for k in range(KH):
                                nc.tensor.matmul(
                                    pt,
                                    s_HT32[:, k, 4 * m:4 * (m + 1), :]
                                        .rearrange("q t b -> q (t b)"),
                                    s_outWT[:, k, NCK * j:NCK * (j + 1)],
                                    start=(k == 0), stop=(k == KH - 1))
                                # alternate evacuation engine to spread load
                            if j % 2 == 0:
                                nc.vector.tensor_copy(stg[:, NCK * j:NCK * (j + 1)], pt)
                            else:
                                nc.scalar.copy(stg[:, NCK * j:NCK * (j + 1)], pt)
                        nc.sync.dma_start(out=d_out[128 * m:128 * (m + 1), :], in_=stg)

            if _dbg:
                nc.sync.dma_start(out=d_hdbg, in_=s_HT32)

    nc.compile()
    return nc


def _prep_inputs(inputs):
    enc = np.asarray(inputs["encoded"], np.float32)      # [L, B, H]
    hidden = np.asarray(inputs["hidden"], np.float32)    # [1, B, H]
    target = np.asarray(inputs["target"])                # [T, B] int
    emb = np.asarray(inputs["emb"], np.float32)          # [V, H]
    attn_W = np.asarray(inputs["attn_W"], np.float32)    # [L, 2H]
    comb_W = np.asarray(inputs["comb_W"], np.float32)    # [H, 2H]
    W_ih = np.asarray(inputs["W_ih"], np.float32)        # [3H, H]
    W_hh = np.asarray(inputs["W_hh"], np.float32)        # [3H, H]
    out_W = np.asarray(inputs["out_W"], np.float32)      # [V, H]
    for bname in ("attn_b", "comb_b", "b_ih", "b_hh", "out_b"):
        assert np.abs(np.asarray(inputs[bname])).max() == 0.0, \
            f"nonzero bias {bname} not supported"

    tokens = np.concatenate(
        [np.full((1, B), SOS, target.dtype), target[:-1]], axis=0)  # [T, B]
    emb_seq = emb[tokens.reshape(-1).astype(np.int64)]              # [T*B, H]
    embT = _pack_k(np.ascontiguousarray(emb_seq.T), KH).astype(ml_dtypes.bfloat16)

    WeT = _pack_k(np.ascontiguousarray(attn_W[:, :H].T), KH).astype(ml_dtypes.bfloat16)
    WhT = _pack_k(np.ascontiguousarray(attn_W[:, H:].T), KH).astype(ml_dtypes.bfloat16)
    combT = _pack_kM(np.ascontiguousarray(comb_W.T), 2 * KH, KH).astype(ml_dtypes.bfloat16)
    WihT = _pack_kM(np.ascontiguousarray(W_ih.T), KH, 3 * KH).astype(ml_dtypes.bfloat16)
    W_hh2 = W_hh.copy()
    W_hh2[2 * H:] *= 0.5
    WhhT = _pack_kM(np.ascontiguousarray(W_hh2.T), KH, 3 * KH).astype(ml_dtypes.bfloat16)

    # einsum stationary: encp[(l + 64*half), p, c, m] = enc[l, 2p+half, 128c+m]
    e5 = enc.reshape(L, B // 2, 2, KH, 128)
    encp = np.ascontiguousarray(
        e5.transpose(2, 0, 1, 3, 4).reshape(128, B // 2, KH, 128)
    ).astype(ml_dtypes.bfloat16)

    h0T = np.ascontiguousarray(hidden[0].T)              # [H, B]
    h0T32 = _pack_k(h0T, KH)
    h0Tbf = h0T32.astype(ml_dtypes.bfloat16)

    base = dict(embT=embT, WeT=WeT, WhT=WhT, combT=combT, WihT=WihT,
                WhhT=WhhT, encp=encp, h0T32=h0T32, h0Tbf=h0Tbf)
    in_maps = []
    for c in range(NCORES):
        m = dict(base)
        wc = np.ascontiguousarray(out_W[c * VS:(c + 1) * VS].T)  # [H, VS]
        m["outWT"] = _pack_k(wc, KH)
        in_maps.append(m)
    return in_maps


def _get_runner():
    import os as _os
    _key = ("runner", _os.environ.get("KREPS", "1"), _os.environ.get("KABL", ""),
            _os.environ.get("KDBG", "0"))
    if _key in _CACHE:
        return _CACHE[_key]
    import jax
    from jax.sharding import Mesh, PartitionSpec, NamedSharding
    try:
        from jax.experimental.shard_map import shard_map
    except ImportError:
        from jax.shard_map import shard_map
    from concourse import bass2jax
    import concourse.mybir as mb

    nc = _build()
    bass2jax.install_neuronx_cc_hook()

    part_name = (nc.partition_id_tensor.name
                 if nc.partition_id_tensor else None)
    in_names, out_names, out_avals = [], [], []
    for alloc in nc.m.functions[0].allocations:
        if not isinstance(alloc, mb.MemoryLocationSet):
            continue
        name = alloc.memorylocations[0].name
        if alloc.kind == "ExternalInput":
            if name != part_name:
                in_names.append(name)
        elif alloc.kind == "ExternalOutput":
            out_names.append(name)
            shape = tuple(alloc.tensor_shape)
            dtype = mb.dt.np(alloc.dtype)
            out_avals.append(jax.core.ShapedArray(shape, dtype))
    n_params = len(in_names)
    all_names = in_names + out_names
    if part_name is not None:
        all_names = all_names + [part_name]

    def _body(*args):
        operands = list(args)
        if part_name is not None:
            operands.append(bass2jax.partition_id_tensor())
        outs = bass2jax._bass_exec_p.bind(
            *operands,
            out_avals=tuple(out_avals),
            in_names=tuple(all_names),
            out_names=tuple(out_names),
            lowering_input_output_aliases=(),
            sim_require_finite=True,
            sim_require_nnan=True,
            nc=nc,
        )
        return tuple(outs)

    devices = jax.devices()[:NCORES]
    mesh = Mesh(np.asarray(devices), ("core",))
    sh = NamedSharding(mesh, PartitionSpec("core"))
    nin = n_params + len(out_names)
    # No donation: the kernel writes every element of every output, so the
    # pre-zeroed output operands are inert and can live on-device forever.
    sharded = jax.jit(
        shard_map(_body, mesh=mesh,
                  in_specs=(PartitionSpec("core"),) * nin,
                  out_specs=(PartitionSpec("core"),) * len(out_names),
                  check_rep=False),
        keep_unused=True,
    )

    # Output-operand placeholders created directly on device (never
    # transferred, never donated, reused every call).
    zkey = ("zeros", tuple((a.shape, str(a.dtype)) for a in out_avals))
    if zkey not in _CACHE:
        _CACHE[zkey] = [
            jax.jit(lambda s=a.shape, d=a.dtype: jax.numpy.zeros(
                (NCORES * s[0], *s[1:]), d), out_shardings=sh)()
            for a in out_avals
        ]
        jax.block_until_ready(_CACHE[zkey])
    dev_zeros = _CACHE[zkey]

    def _dev_inputs(in_maps):
        """Device-resident concatenated inputs, cached per in_maps object."""
        ck = ("devin", id(in_maps))
        hit = _CACHE.get(ck)
        if hit is not None and hit[0] is in_maps:
            return hit[1]
        concat_in = [
            np.concatenate([np.asarray(in_maps[c][nm]) for c in range(NCORES)],
                           axis=0)
            for nm in in_names
        ]
        dev_in = [jax.device_put(a, sh) for a in concat_in]
        jax.block_until_ready(dev_in)
        _CACHE[ck] = (in_maps, dev_in)
        return dev_in

    def runner(in_maps, pull=True):
        dev_in = _dev_inputs(in_maps)
        out_arrs = sharded(*dev_in, *dev_zeros)
        if not pull:
            jax.block_until_ready(out_arrs)
            return None
        oarr = out_arrs[out_names.index("o")]
        shards = [s.data for s in oarr.addressable_shards]
        for s in shards:
            s.copy_to_host_async()
        o = [np.asarray(s) for s in shards]        # NCORES × [TB, VS] fp16
        if "hdbg" in out_names:
            h = np.asarray(out_arrs[out_names.index("hdbg")])
            _CACHE["hdbg"] = h.reshape(NCORES, 128, KH, T, B)[0]
        return o

    _CACHE[_key] = runner
    return runner


def _assemble(o) -> np.ndarray:
    """NCORES × [TB, VS] fp16 shards -> [T, B, V] f32."""
    from concurrent.futures import ThreadPoolExecutor
    out = np.empty((T, B, V), np.float32)
    ov = out.reshape(T, B, NCORES, VS)

    def put(c):
        ov[:, :, c, :] = o[c].reshape(T, B, VS)

    with ThreadPoolExecutor(NCORES) as ex:
        list(ex.map(put, range(NCORES)))
    return out


def kernel(**inputs) -> np.ndarray:
    in_maps = _prep_inputs(inputs)
    o = _get_runner()(in_maps)               # NCORES x [T*B, VS] fp16
    return _assemble(o)

